# revision 9
# baseline (speedup 1.0000x reference)
"""Multi-head attention with RoPE (LLaMA-style) on 8 Trainium2 NeuronCores.

Head-parallel tensor parallelism: each core computes 2 of 16 heads
(projections + flash-style attention) and a partial output projection over
the full sequence; the host sums the 8 per-core partials (the all-reduce)
and adds wo_b plus the folded v-bias term.

Layout: q/k rows are pair-split per head ([evens(32)|odds(32)] per head) so
each head's QK matmul is one 64-row PE group; the two heads' matmuls run
concurrently on disjoint row groups into one [128,2,512] scores tile, and
one exp instruction covers both heads.  q/k/v projections are interleaved
with attention (PSUM: 2 proj banks + 4 scores + 2 ppv = 8); the output
projection runs as a tail with bf16 partials.

Softmax denominators come from an appended ones-column in v; reciprocal is
exp(-ln d) on the scalar engine (same activation table set as the softmax
exp), broadcast across partitions by a selection-matrix matmul.

Self-contained: hardcodes B=1, S=4096, D=1024, H=16, HD=64, 8 cores.
"""

import sys
import types

import ml_dtypes
import numpy as np

B, S, D, H, HD = 1, 4096, 1024, 16, 64
HALF = HD // 2
NC = 8                    # cores
HPC = H // NC             # heads per core (2)
CPC = HPC * HD            # qkv dims per core (128)
QCH = 512                 # query chunk (free dim of scores matmuls)
KCH = 128                 # key chunk (partition dim of scores matmuls)
NQC = S // QCH            # 8 query chunks
NKC = S // KCH            # 32 key chunks
P = 128
KC = D // P               # 8 contraction chunks for projections
VW = 2 * (HD + 1) + 2     # v_sb row width (hd|1 per head + pad, 4B aligned)


def _install_ntff_shim():
    """antenv.axon_hooks isn't injected in this image; recreate it so
    run_bass_kernel_spmd(trace=True) can capture NTFF profiles."""
    if "antenv.axon_hooks" in sys.modules:
        return
    try:
        from trn_agent_boot.trn_boot import _ntff_profile_via_ctypes

        hook = _ntff_profile_via_ctypes("/opt/axon/libaxon_pjrt.so")
    except Exception:
        hook = None
    mod = types.ModuleType("antenv.axon_hooks")
    mod.get_axon_ntff_profile_hook = lambda: hook
    sys.modules["antenv.axon_hooks"] = mod


_install_ntff_shim()

import concourse.bacc as bacc  # noqa: E402
import concourse.mybir as mybir  # noqa: E402
import concourse.tile as tile  # noqa: E402
from concourse.bass_utils import run_bass_kernel_spmd  # noqa: E402

F32 = mybir.dt.float32
BF16 = mybir.dt.bfloat16
AX = mybir.AluOpType
EXP = mybir.ActivationFunctionType.Exp
LOG = mybir.ActivationFunctionType.Ln

_BUILD_CACHE: dict = {}


def _build(mask_mode: str):
    """Build the per-core Bass program.  mask_mode: causal | none | general."""
    if mask_mode in _BUILD_CACHE:
        return _BUILD_CACHE[mask_mode]

    nc = bacc.Bacc("TRN2", target_bir_lowering=False, debug=False, num_devices=NC)

    xT = nc.dram_tensor("xT", [D, S], BF16, kind="ExternalInput")
    wqkvT = nc.dram_tensor("wqkvT", [D, 3 * CPC], BF16, kind="ExternalInput")
    woT = nc.dram_tensor("woT", [CPC, D], BF16, kind="ExternalInput")
    # trigc rows: cosT tiled 4x vertically; trigs: sinT tiled 4x
    trigc = nc.dram_tensor("trigc", [P, S], F32, kind="ExternalInput")
    trigs = nc.dram_tensor("trigs", [P, S], F32, kind="ExternalInput")
    qkb = nc.dram_tensor("qkb", [P, 2], F32, kind="ExternalInput")
    tri = None
    maskT = None
    if mask_mode == "causal":
        tri = nc.dram_tensor("tri", [KCH, KCH], BF16, kind="ExternalInput")
    elif mask_mode == "general":
        maskT = nc.dram_tensor("maskT", [S, S], F32, kind="ExternalInput")
    y_out = nc.dram_tensor("y", [S, D], BF16, kind="ExternalOutput")

    causal = mask_mode == "causal"

    def nj_of(qc):
        return 4 * (qc + 1) if causal else NKC

    with tile.TileContext(nc) as tc:
        with tc.tile_pool(name="consts", bufs=1) as cpool:
            qT = cpool.tile([P, S], BF16)   # [2 heads x (ev32|od32), s]
            kT = cpool.tile([P, S], BF16)
            v_sb = cpool.tile([P, NKC, VW], BF16)  # [s%128, s//128, hd|1 x2]
            attnT = cpool.tile([P, S], BF16)
            w_sb = cpool.tile([P, KC, 3 * CPC], BF16)
            nc.sync.dma_start(
                out=w_sb[:], in_=wqkvT.ap().rearrange("(a p) c -> p a c", p=P)
            )
            qkb_sb = cpool.tile([P, 2], F32)
            nc.sync.dma_start(out=qkb_sb[:], in_=qkb.ap())
            sel_sb = cpool.tile([33, CPC], F32)
            nc.vector.memset(sel_sb[:], 0.0)
            nc.vector.memset(sel_sb[0:1, 0:HD], 1.0)
            nc.vector.memset(sel_sb[32:33, HD:CPC], 1.0)
            nc.vector.memset(v_sb[:, :, HD : HD + 1], 1.0)
            c1 = HD + 1 + HD  # second ones column
            nc.vector.memset(v_sb[:, :, c1 : c1 + 1], 1.0)
            if causal:
                tri_sb = cpool.tile([KCH, KCH], BF16)
                nc.sync.dma_start(out=tri_sb[:], in_=tri.ap())
            woT_sb = cpool.tile([CPC, D], BF16)
            trigc_sb = cpool.tile([P, S], F32)
            trigs_sb = cpool.tile([P, S], F32)
            xT_sb = cpool.tile([P, KC, S], BF16)
            nc.sync.dma_start(
                out=xT_sb[:, :, 0:QCH],
                in_=xT.ap()[:, 0:QCH].rearrange("(a p) s -> p a s", p=P),
            )
            nc.sync.dma_start(out=trigc_sb[:], in_=trigc.ap())
            nc.sync.dma_start(out=trigs_sb[:], in_=trigs.ap())
            for sc in range(1, NQC):
                ssl = slice(sc * QCH, (sc + 1) * QCH)
                nc.sync.dma_start(
                    out=xT_sb[:, :, ssl],
                    in_=xT.ap()[:, ssl].rearrange("(a p) s -> p a s", p=P),
                )
            nc.sync.dma_start(out=woT_sb[:], in_=woT.ap())

            # ---- q/k/v projections + RoPE interleaved with attention ----
            with (
                tc.tile_pool(name="pqk", bufs=2, space="PSUM") as pqk_pool,
                tc.tile_pool(name="sc", bufs=2, space="PSUM") as sc_pool,
                tc.tile_pool(name="ppv", bufs=2, space="PSUM") as ppv_pool,
                tc.tile_pool(name="pt", bufs=4) as pt_pool,
                tc.tile_pool(name="tp", bufs=2) as t_pool,
                tc.tile_pool(name="dn", bufs=4) as dn_pool,
                tc.tile_pool(name="pvs", bufs=2) as pv_sb_pool,
                tc.tile_pool(name="mload", bufs=4) as mload_pool,
            ):
                pending = []  # deferred bc+normalize closures

                def emit_proj(sc):
                    ssl = slice(sc * QCH, (sc + 1) * QCH)
                    for dst, bcol in ((qT, 0), (kT, 1)):
                        ps = pqk_pool.tile([P, QCH], F32, tag="pqk", name="ps")
                        for kc in range(KC):
                            nc.tensor.matmul(
                                ps[:],
                                lhsT=w_sb[:, kc, bcol * CPC : (bcol + 1) * CPC],
                                rhs=xT_sb[:, kc, ssl],
                                start=(kc == 0),
                                stop=(kc == KC - 1),
                            )
                        t = t_pool.tile([P, QCH], F32)
                        nc.vector.scalar_tensor_tensor(
                            t[:], ps[:], qkb_sb[:, bcol : bcol + 1],
                            trigc_sb[:, ssl], op0=AX.add, op1=AX.mult,
                        )
                        u = sc_pool.tile([P, QCH], F32, tag="st", name="u")
                        nc.vector.scalar_tensor_tensor(
                            u[:], ps[:], qkb_sb[:, bcol : bcol + 1],
                            trigs_sb[:, ssl], op0=AX.add, op1=AX.mult,
                        )
                        for h in range(HPC):
                            ev = slice(64 * h, 64 * h + 32)
                            od = slice(64 * h + 32, 64 * h + 64)
                            nc.vector.tensor_sub(dst[ev, ssl], t[ev, :], u[od, :])
                            nc.vector.tensor_add(dst[od, ssl], t[od, :], u[ev, :])

                def emit_v(sc):
                    # v rides in the proj psum pool: 4 row-blocks, one bank
                    psv = pqk_pool.tile([P, 4, CPC], F32, tag="pqk", name="psv")
                    for j4 in range(4):
                        sb = sc * 4 + j4
                        for kc in range(KC):
                            nc.tensor.matmul(
                                psv[:, j4, :],
                                lhsT=xT_sb[:, kc, sb * P : (sb + 1) * P],
                                rhs=w_sb[:, kc, 2 * CPC : 3 * CPC],
                                start=(j4 == 0 and kc == 0),
                                stop=(j4 == 3 and kc == KC - 1),
                                skip_group_check=True,
                            )
                    csl = slice(sc * 4, sc * 4 + 4)
                    nc.vector.tensor_copy(v_sb[:, csl, 0:HD], psv[:, :, 0:HD])
                    nc.vector.tensor_copy(
                        v_sb[:, csl, HD + 1 : HD + 1 + HD], psv[:, :, HD:CPC]
                    )

                def make_bc_norm(qc, pv, rec):
                    def emit():
                        qsl = slice(qc * QCH, (qc + 1) * QCH)
                        bc_ps = sc_pool.tile([P, QCH], F32, tag="st", name="bc")
                        nc.tensor.matmul(
                            bc_ps[:], lhsT=sel_sb[:], rhs=rec[:],
                            start=True, stop=True, skip_group_check=True,
                        )
                        for h in range(HPC):
                            hr = slice(64 * h, 64 * h + 64)
                            nc.vector.tensor_mul(
                                attnT[hr, qsl], pv[hr, :], bc_ps[hr, :]
                            )

                    return emit

                def emit_attn(qc):
                    nj = nj_of(qc)
                    q0 = qc * QCH
                    ppv0 = ppv_pool.tile([HD + 1, QCH], F32, tag="ppv", name="ppv0")
                    ppv1 = ppv_pool.tile([HD + 1, QCH], F32, tag="ppv", name="ppv1")
                    prevs = []

                    def emit_pv(ent):
                        j, pt, lo = ent
                        for h, ppv in ((0, ppv0), (1, ppv1)):
                            nc.tensor.matmul(
                                ppv[:, lo:QCH],
                                lhsT=v_sb[
                                    :, j, h * (HD + 1) : (h + 1) * (HD + 1)
                                ],
                                rhs=pt[:, h, lo:QCH],
                                start=(j == 0),
                                stop=(j == nj - 1),
                                skip_group_check=True,
                            )

                    for j in range(nj):
                        lo = max(0, KCH * j - q0) if causal else 0
                        diag = causal and KCH * j >= q0
                        st = sc_pool.tile([P, 2, QCH], F32, tag="st", name="st")
                        for h in range(HPC):
                            hr = slice(64 * h, 64 * h + 64)
                            nc.tensor.matmul(
                                st[:, h, lo:QCH],
                                lhsT=kT[hr, j * KCH : (j + 1) * KCH],
                                rhs=qT[hr, q0 + lo : q0 + QCH],
                                start=True,
                                stop=True,
                                skip_group_check=True,
                            )
                        if j == 3 and pending:
                            pending.pop(0)()
                        if len(prevs) == 2:
                            emit_pv(prevs.pop(0))
                        if mask_mode == "general":
                            mt = mload_pool.tile([KCH, QCH], F32)
                            nc.sync.dma_start(
                                out=mt[:],
                                in_=maskT.ap()[
                                    j * KCH : (j + 1) * KCH, q0 : q0 + QCH
                                ],
                            )
                            for h in range(HPC):
                                nc.vector.tensor_add(
                                    st[:, h, :], st[:, h, :], mt[:]
                                )
                        pt = pt_pool.tile([P, 2, QCH], BF16)
                        nc.scalar.activation(
                            pt[:, :, lo:QCH], st[:, :, lo:QCH], EXP, scale=0.125
                        )
                        if diag:
                            # multiplicative 0/1 mask after the exp: cheap
                            # (bf16 2x) and two j-periods off the PV chain
                            for h in range(HPC):
                                nc.vector.tensor_mul(
                                    pt[:, h, lo : lo + KCH],
                                    pt[:, h, lo : lo + KCH],
                                    tri_sb[:],
                                )
                        prevs.append((j, pt, lo))
                    for ent in prevs:
                        emit_pv(ent)

                    # normalization prep; reciprocal as exp(-ln d) on the
                    # scalar engine (same table set as the softmax exp)
                    dn = dn_pool.tile([33, QCH], F32, tag="dn", name="dn")
                    nc.vector.memset(dn[:], 1.0)
                    nc.vector.tensor_copy(dn[0:1, :], ppv0[HD : HD + 1, :])
                    nc.vector.tensor_copy(dn[32:33, :], ppv1[HD : HD + 1, :])
                    pv = pv_sb_pool.tile([P, QCH], F32)
                    nc.vector.tensor_copy(pv[0:HD, :], ppv0[0:HD, :])
                    nc.vector.tensor_copy(pv[HD:P, :], ppv1[0:HD, :])
                    rec = dn_pool.tile([33, QCH], F32, tag="rec", name="rec")
                    nc.vector.reciprocal(rec[:], dn[:])
                    pending.append(make_bc_norm(qc, pv, rec))

                if causal:
                    emit_proj(0)
                    emit_v(0)
                    for qc in range(NQC):
                        if qc + 1 < NQC:
                            emit_proj(qc + 1)
                        emit_attn(qc)
                        if qc + 1 < NQC:
                            emit_v(qc + 1)
                else:
                    for sc in range(NQC):
                        emit_proj(sc)
                        emit_v(sc)
                    for qc in range(NQC):
                        emit_attn(qc)
                for fn in pending:
                    fn()
                pending.clear()

            # ---------------- output projection tail ----------------
            with (
                tc.tile_pool(name="py", bufs=4, space="PSUM") as py_pool,
                tc.tile_pool(name="ysb", bufs=4) as y_pool,
            ):
                for sb in range(S // P):
                    for dh in range(2):
                        psy = py_pool.tile([P, QCH], F32)
                        nc.tensor.matmul(
                            psy[:],
                            lhsT=attnT[:, sb * P : (sb + 1) * P],
                            rhs=woT_sb[:, dh * QCH : (dh + 1) * QCH],
                            start=True,
                            stop=True,
                        )
                        ysb = y_pool.tile([P, QCH], BF16)
                        if (sb * 2 + dh) % 2 == 0:
                            nc.scalar.copy(ysb[:], psy[:])
                        else:
                            nc.vector.tensor_copy(ysb[:], psy[:])
                        nc.sync.dma_start(
                            out=y_out.ap()[
                                sb * P : (sb + 1) * P, dh * QCH : (dh + 1) * QCH
                            ],
                            in_=ysb[:],
                        )

    nc.compile()
    _BUILD_CACHE[mask_mode] = nc
    return nc


def _detect_mask_mode(mask: np.ndarray):
    m = np.asarray(mask, np.float32).reshape(S, S)
    if not m.any():
        return "none", 0.0, m
    mval = float(m[0, 1])
    if mval < -1e8 and np.array_equal(
        m, np.triu(np.full((S, S), mval, np.float32), 1)
    ):
        return "causal", mval, m
    return "general", 0.0, m


def kernel(
    x, start_pos, freqs_cos, freqs_sin, mask,
    wq_w, wq_b, wk_w, wk_b, wv_w, wv_b, wo_w, wo_b,
):
    x = np.asarray(x, np.float32).reshape(S, D)
    freqs_cos = np.asarray(freqs_cos, np.float32)
    freqs_sin = np.asarray(freqs_sin, np.float32)
    mask_mode, mval, m2d = _detect_mask_mode(np.asarray(mask))

    # pair-split permutation within each head: [0,2,..,62, 1,3,..,63]
    perm1 = np.concatenate([np.arange(0, HD, 2), np.arange(1, HD, 2)])
    perm = np.concatenate([perm1 + h * HD for h in range(HPC)])

    xT_bf = np.ascontiguousarray(x.T).astype(ml_dtypes.bfloat16)

    cosT = np.ascontiguousarray(freqs_cos.T)  # [32, S]
    sinT = np.ascontiguousarray(freqs_sin.T)
    trigc = np.tile(cosT, (4, 1)).astype(np.float32)
    trigs = np.tile(sinT, (4, 1)).astype(np.float32)

    woT_full = np.ascontiguousarray(np.asarray(wo_w, np.float32).T)

    tri_np = None
    if mask_mode == "causal":
        kk = np.arange(KCH)[:, None]
        qq = np.arange(KCH)[None, :]
        tri_np = np.where(kk > qq, 0.0, 1.0).astype(ml_dtypes.bfloat16)
    maskT_np = None
    if mask_mode == "general":
        maskT_np = np.ascontiguousarray((8.0 * m2d).T.astype(np.float32))

    in_maps = []
    for c in range(NC):
        rows = slice(c * CPC, (c + 1) * CPC)
        wq_s = np.asarray(wq_w, np.float32)[rows, :][perm, :]
        wk_s = np.asarray(wk_w, np.float32)[rows, :][perm, :]
        wv_s = np.asarray(wv_w, np.float32)[rows, :]
        wqkvT = np.concatenate([wq_s.T, wk_s.T, wv_s.T], axis=1).astype(
            ml_dtypes.bfloat16
        )
        qb = np.asarray(wq_b, np.float32)[rows][perm]
        kb = np.asarray(wk_b, np.float32)[rows][perm]
        im = {
            "xT": xT_bf,
            "wqkvT": np.ascontiguousarray(wqkvT),
            "woT": np.ascontiguousarray(woT_full[rows, :]).astype(ml_dtypes.bfloat16),
            "trigc": trigc,
            "trigs": trigs,
            "qkb": np.stack([qb, kb], axis=1).astype(np.float32),
        }
        if mask_mode == "causal":
            im["tri"] = tri_np
        elif mask_mode == "general":
            im["maskT"] = maskT_np
        in_maps.append(im)

    nc = _build(mask_mode)
    res = run_bass_kernel_spmd(nc, in_maps, list(range(NC)))
    y = np.zeros((S, D), np.float64)
    for c in range(NC):
        y += np.asarray(res.results[c]["y"]).astype(np.float64)
    y += np.asarray(wo_b, np.float64)
    # v-bias folded out of the device kernel: softmax rows sum to 1, so the
    # bias contributes exactly wv_b @ wo_w.T to every output row
    y += np.asarray(wv_b, np.float64) @ np.asarray(wo_w, np.float64).T
    return y.reshape(B, S, D).astype(np.float32)


# revision 10
# speedup vs baseline: 1.0404x; 1.0404x over previous
"""Multi-head attention with RoPE (LLaMA-style) on 8 Trainium2 NeuronCores.

Head-parallel tensor parallelism: each core computes 2 of 16 heads
(projections + flash-style attention) and a partial output projection over
the full sequence; the host sums the 8 per-core partials (the all-reduce)
and adds wo_b plus the folded v-bias term.

Layout: q/k rows are pair-split per head ([evens(32)|odds(32)] per head) so
each head's QK matmul is one 64-row PE group; the two heads' matmuls run
concurrently on disjoint row groups into one [128,2,512] scores tile, and
one exp instruction covers both heads.  q/k/v projections are interleaved
with attention (PSUM: 2 proj banks + 4 scores + 2 ppv = 8); the output
projection runs as a tail with bf16 partials.

Softmax denominators come from an appended ones-column in v; reciprocal is
exp(-ln d) on the scalar engine (same activation table set as the softmax
exp), broadcast across partitions by a selection-matrix matmul.

Self-contained: hardcodes B=1, S=4096, D=1024, H=16, HD=64, 8 cores.
"""

import sys
import types

import ml_dtypes
import numpy as np

B, S, D, H, HD = 1, 4096, 1024, 16, 64
HALF = HD // 2
NC = 8                    # cores
HPC = H // NC             # heads per core (2)
CPC = HPC * HD            # qkv dims per core (128)
QCH = 512                 # query chunk (free dim of scores matmuls)
KCH = 128                 # key chunk (partition dim of scores matmuls)
NQC = S // QCH            # 8 query chunks
NKC = S // KCH            # 32 key chunks
P = 128
KC = D // P               # 8 contraction chunks for projections
VW = 2 * (HD + 1) + 2     # v_sb row width (hd|1 per head + pad, 4B aligned)


def _install_ntff_shim():
    """antenv.axon_hooks isn't injected in this image; recreate it so
    run_bass_kernel_spmd(trace=True) can capture NTFF profiles."""
    if "antenv.axon_hooks" in sys.modules:
        return
    try:
        from trn_agent_boot.trn_boot import _ntff_profile_via_ctypes

        hook = _ntff_profile_via_ctypes("/opt/axon/libaxon_pjrt.so")
    except Exception:
        hook = None
    mod = types.ModuleType("antenv.axon_hooks")
    mod.get_axon_ntff_profile_hook = lambda: hook
    sys.modules["antenv.axon_hooks"] = mod


_install_ntff_shim()

import concourse.bacc as bacc  # noqa: E402
import concourse.mybir as mybir  # noqa: E402
import concourse.tile as tile  # noqa: E402
from concourse.bass_utils import run_bass_kernel_spmd  # noqa: E402

F32 = mybir.dt.float32
BF16 = mybir.dt.bfloat16
AX = mybir.AluOpType
EXP = mybir.ActivationFunctionType.Exp
LOG = mybir.ActivationFunctionType.Ln

_BUILD_CACHE: dict = {}


def _build(mask_mode: str):
    """Build the per-core Bass program.  mask_mode: causal | none | general."""
    if mask_mode in _BUILD_CACHE:
        return _BUILD_CACHE[mask_mode]

    nc = bacc.Bacc("TRN2", target_bir_lowering=False, debug=False, num_devices=NC)

    xT = nc.dram_tensor("xT", [D, S], BF16, kind="ExternalInput")
    wqkvT = nc.dram_tensor("wqkvT", [D, 3 * CPC], BF16, kind="ExternalInput")
    woT = nc.dram_tensor("woT", [CPC, D], BF16, kind="ExternalInput")
    # trigc rows: cosT tiled 4x vertically; trigs: sinT tiled 4x
    trigc = nc.dram_tensor("trigc", [P, S], F32, kind="ExternalInput")
    trigs = nc.dram_tensor("trigs", [P, S], F32, kind="ExternalInput")
    qkb = nc.dram_tensor("qkb", [P, 2], F32, kind="ExternalInput")
    tri = None
    maskT = None
    if mask_mode == "causal":
        tri = nc.dram_tensor("tri", [KCH, KCH], BF16, kind="ExternalInput")
    elif mask_mode == "general":
        maskT = nc.dram_tensor("maskT", [S, S], F32, kind="ExternalInput")
    y_out = nc.dram_tensor("y", [S, D], BF16, kind="ExternalOutput")

    causal = mask_mode == "causal"

    def nj_of(qc):
        return 4 * (qc + 1) if causal else NKC

    with tile.TileContext(nc) as tc:
        with tc.tile_pool(name="consts", bufs=1) as cpool:
            qT = cpool.tile([P, S], BF16)   # [2 heads x (ev32|od32), s]
            kT = cpool.tile([P, S], BF16)
            v_sb = cpool.tile([P, NKC, VW], BF16)  # [s%128, s//128, hd|1 x2]
            attnT = cpool.tile([P, S], BF16)
            w_sb = cpool.tile([P, KC, 3 * CPC], BF16)
            nc.sync.dma_start(
                out=w_sb[:], in_=wqkvT.ap().rearrange("(a p) c -> p a c", p=P)
            )
            qkb_sb = cpool.tile([P, 2], F32)
            nc.sync.dma_start(out=qkb_sb[:], in_=qkb.ap())
            sel_sb = cpool.tile([33, CPC], F32)
            nc.vector.memset(sel_sb[:], 0.0)
            nc.vector.memset(sel_sb[0:1, 0:HD], 1.0)
            nc.vector.memset(sel_sb[32:33, HD:CPC], 1.0)
            nc.vector.memset(v_sb[:, :, HD : HD + 1], 1.0)
            c1 = HD + 1 + HD  # second ones column
            nc.vector.memset(v_sb[:, :, c1 : c1 + 1], 1.0)
            if causal:
                tri_sb = cpool.tile([KCH, KCH], BF16)
                nc.sync.dma_start(out=tri_sb[:], in_=tri.ap())
            woT_sb = cpool.tile([CPC, D], BF16)
            trigc_sb = cpool.tile([P, S], F32)
            trigs_sb = cpool.tile([P, S], F32)
            xT_sb = cpool.tile([P, KC, S], BF16)
            nc.sync.dma_start(
                out=xT_sb[:, :, 0:QCH],
                in_=xT.ap()[:, 0:QCH].rearrange("(a p) s -> p a s", p=P),
            )
            nc.sync.dma_start(out=trigc_sb[:], in_=trigc.ap())
            nc.sync.dma_start(out=trigs_sb[:], in_=trigs.ap())
            for sc in range(1, NQC):
                ssl = slice(sc * QCH, (sc + 1) * QCH)
                nc.sync.dma_start(
                    out=xT_sb[:, :, ssl],
                    in_=xT.ap()[:, ssl].rearrange("(a p) s -> p a s", p=P),
                )
            nc.sync.dma_start(out=woT_sb[:], in_=woT.ap())

            # ---- q/k/v projections + RoPE interleaved with attention ----
            with (
                tc.tile_pool(name="pqk", bufs=2, space="PSUM") as pqk_pool,
                tc.tile_pool(name="sc", bufs=2, space="PSUM") as sc_pool,
                tc.tile_pool(name="ppv", bufs=2, space="PSUM") as ppv_pool,
                tc.tile_pool(name="pt", bufs=4) as pt_pool,
                tc.tile_pool(name="tp", bufs=2) as t_pool,
                tc.tile_pool(name="dn", bufs=4) as dn_pool,
                tc.tile_pool(name="pvs", bufs=2) as pv_sb_pool,
                tc.tile_pool(name="mload", bufs=4) as mload_pool,
            ):
                pending = []  # deferred bc+normalize closures

                def proj_units(sc):
                    """q/k/v projection + RoPE for chunk sc as small closures
                    that interleave into the attention j-loop (so the PE FIFO
                    never holds a long proj block ahead of the QK->exp chain).
                    """
                    ssl = slice(sc * QCH, (sc + 1) * QCH)
                    units = []
                    box = {}

                    def qk_mms(bcol, klo, khi, first, last):
                        def emit():
                            if first:
                                box[bcol] = pqk_pool.tile(
                                    [P, QCH], F32, tag="pqk", name="ps"
                                )
                            ps = box[bcol]
                            for kc in range(klo, khi):
                                nc.tensor.matmul(
                                    ps[:],
                                    lhsT=w_sb[
                                        :, kc, bcol * CPC : (bcol + 1) * CPC
                                    ],
                                    rhs=xT_sb[:, kc, ssl],
                                    start=(kc == 0),
                                    stop=(kc == KC - 1),
                                    skip_group_check=True,
                                )
                        return emit

                    def rope(dst, bcol):
                        def emit():
                            ps = box[bcol]
                            t = t_pool.tile([P, QCH], F32)
                            nc.vector.scalar_tensor_tensor(
                                t[:], ps[:], qkb_sb[:, bcol : bcol + 1],
                                trigc_sb[:, ssl], op0=AX.add, op1=AX.mult,
                            )
                            u = sc_pool.tile([P, QCH], F32, tag="st", name="u")
                            nc.vector.scalar_tensor_tensor(
                                u[:], ps[:], qkb_sb[:, bcol : bcol + 1],
                                trigs_sb[:, ssl], op0=AX.add, op1=AX.mult,
                            )
                            for h in range(HPC):
                                ev = slice(64 * h, 64 * h + 32)
                                od = slice(64 * h + 32, 64 * h + 64)
                                nc.vector.tensor_sub(
                                    dst[ev, ssl], t[ev, :], u[od, :]
                                )
                                nc.vector.tensor_add(
                                    dst[od, ssl], t[od, :], u[ev, :]
                                )
                        return emit

                    def v_mms(j4):
                        def emit():
                            if j4 == 0:
                                box["v"] = pqk_pool.tile(
                                    [P, 4, CPC], F32, tag="pqk", name="psv"
                                )
                            psv = box["v"]
                            sb = sc * 4 + j4
                            for kc in range(KC):
                                nc.tensor.matmul(
                                    psv[:, j4, :],
                                    lhsT=xT_sb[:, kc, sb * P : (sb + 1) * P],
                                    rhs=w_sb[:, kc, 2 * CPC : 3 * CPC],
                                    start=(j4 == 0 and kc == 0),
                                    stop=(j4 == 3 and kc == KC - 1),
                                    skip_group_check=True,
                                )
                        return emit

                    def v_copy():
                        psv = box["v"]
                        csl = slice(sc * 4, sc * 4 + 4)
                        nc.vector.tensor_copy(
                            v_sb[:, csl, 0:HD], psv[:, :, 0:HD]
                        )
                        nc.vector.tensor_copy(
                            v_sb[:, csl, HD + 1 : HD + 1 + HD], psv[:, :, HD:CPC]
                        )

                    units.append(qk_mms(0, 0, 4, True, False))
                    units.append(qk_mms(0, 4, KC, False, True))
                    units.append(qk_mms(1, 0, 4, True, False))
                    units.append(qk_mms(1, 4, KC, False, True))
                    units.append(rope(qT, 0))
                    units.append(rope(kT, 1))
                    for j4 in range(4):
                        units.append(v_mms(j4))
                    units.append(v_copy)
                    return units

                def emit_proj(sc):
                    for u in proj_units(sc):
                        u()

                def make_bc_norm(qc, pv, rec):
                    def emit():
                        qsl = slice(qc * QCH, (qc + 1) * QCH)
                        bc_ps = sc_pool.tile([P, QCH], F32, tag="st", name="bc")
                        nc.tensor.matmul(
                            bc_ps[:], lhsT=sel_sb[:], rhs=rec[:],
                            start=True, stop=True, skip_group_check=True,
                        )
                        for h in range(HPC):
                            hr = slice(64 * h, 64 * h + 64)
                            nc.vector.tensor_mul(
                                attnT[hr, qsl], pv[hr, :], bc_ps[hr, :]
                            )

                    return emit

                def emit_attn(qc, units=()):
                    units = list(units)
                    nj = nj_of(qc)
                    q0 = qc * QCH
                    ppv0 = ppv_pool.tile([HD + 1, QCH], F32, tag="ppv", name="ppv0")
                    ppv1 = ppv_pool.tile([HD + 1, QCH], F32, tag="ppv", name="ppv1")
                    prevs = []

                    def emit_pv(ent):
                        j, pt, lo = ent
                        for h, ppv in ((0, ppv0), (1, ppv1)):
                            nc.tensor.matmul(
                                ppv[:, lo:QCH],
                                lhsT=v_sb[
                                    :, j, h * (HD + 1) : (h + 1) * (HD + 1)
                                ],
                                rhs=pt[:, h, lo:QCH],
                                start=(j == 0),
                                stop=(j == nj - 1),
                                skip_group_check=True,
                            )

                    for j in range(nj):
                        lo = max(0, KCH * j - q0) if causal else 0
                        diag = causal and KCH * j >= q0
                        st = sc_pool.tile([P, 2, QCH], F32, tag="st", name="st")
                        for h in range(HPC):
                            hr = slice(64 * h, 64 * h + 64)
                            nc.tensor.matmul(
                                st[:, h, lo:QCH],
                                lhsT=kT[hr, j * KCH : (j + 1) * KCH],
                                rhs=qT[hr, q0 + lo : q0 + QCH],
                                start=True,
                                stop=True,
                                skip_group_check=True,
                            )
                        if units and j >= 1:
                            npop = 3 if nj < 8 else 2
                            for _ in range(npop):
                                if units:
                                    units.pop(0)()
                        if j == 3 and pending:
                            pending.pop(0)()
                        if len(prevs) == 2:
                            emit_pv(prevs.pop(0))
                        if mask_mode == "general":
                            mt = mload_pool.tile([KCH, QCH], F32)
                            nc.sync.dma_start(
                                out=mt[:],
                                in_=maskT.ap()[
                                    j * KCH : (j + 1) * KCH, q0 : q0 + QCH
                                ],
                            )
                            for h in range(HPC):
                                nc.vector.tensor_add(
                                    st[:, h, :], st[:, h, :], mt[:]
                                )
                        pt = pt_pool.tile([P, 2, QCH], BF16)
                        nc.scalar.activation(
                            pt[:, :, lo:QCH], st[:, :, lo:QCH], EXP, scale=0.125
                        )
                        if diag:
                            # multiplicative 0/1 mask after the exp: cheap
                            # (bf16 2x) and two j-periods off the PV chain
                            for h in range(HPC):
                                nc.vector.tensor_mul(
                                    pt[:, h, lo : lo + KCH],
                                    pt[:, h, lo : lo + KCH],
                                    tri_sb[:],
                                )
                        prevs.append((j, pt, lo))
                    for u in units:
                        u()
                    for ent in prevs:
                        emit_pv(ent)

                    # normalization prep; reciprocal as exp(-ln d) on the
                    # scalar engine (same table set as the softmax exp)
                    dn = dn_pool.tile([33, QCH], F32, tag="dn", name="dn")
                    nc.vector.memset(dn[:], 1.0)
                    nc.vector.tensor_copy(dn[0:1, :], ppv0[HD : HD + 1, :])
                    nc.vector.tensor_copy(dn[32:33, :], ppv1[HD : HD + 1, :])
                    pv = pv_sb_pool.tile([P, QCH], F32)
                    nc.vector.tensor_copy(pv[0:HD, :], ppv0[0:HD, :])
                    nc.vector.tensor_copy(pv[HD:P, :], ppv1[0:HD, :])
                    rec = dn_pool.tile([33, QCH], F32, tag="rec", name="rec")
                    nc.vector.reciprocal(rec[:], dn[:])
                    pending.append(make_bc_norm(qc, pv, rec))

                if causal:
                    emit_proj(0)
                    for qc in range(NQC):
                        nxt = proj_units(qc + 1) if qc + 1 < NQC else ()
                        emit_attn(qc, nxt)
                else:
                    for sc in range(NQC):
                        emit_proj(sc)
                    for qc in range(NQC):
                        emit_attn(qc)
                for fn in pending:
                    fn()
                pending.clear()

            # ---------------- output projection tail ----------------
            with (
                tc.tile_pool(name="py", bufs=4, space="PSUM") as py_pool,
                tc.tile_pool(name="ysb", bufs=4) as y_pool,
            ):
                for sb in range(S // P):
                    for dh in range(2):
                        psy = py_pool.tile([P, QCH], F32)
                        nc.tensor.matmul(
                            psy[:],
                            lhsT=attnT[:, sb * P : (sb + 1) * P],
                            rhs=woT_sb[:, dh * QCH : (dh + 1) * QCH],
                            start=True,
                            stop=True,
                        )
                        ysb = y_pool.tile([P, QCH], BF16)
                        if (sb * 2 + dh) % 2 == 0:
                            nc.scalar.copy(ysb[:], psy[:])
                        else:
                            nc.vector.tensor_copy(ysb[:], psy[:])
                        nc.sync.dma_start(
                            out=y_out.ap()[
                                sb * P : (sb + 1) * P, dh * QCH : (dh + 1) * QCH
                            ],
                            in_=ysb[:],
                        )

    nc.compile()
    _BUILD_CACHE[mask_mode] = nc
    return nc


def _detect_mask_mode(mask: np.ndarray):
    m = np.asarray(mask, np.float32).reshape(S, S)
    if not m.any():
        return "none", 0.0, m
    mval = float(m[0, 1])
    if mval < -1e8 and np.array_equal(
        m, np.triu(np.full((S, S), mval, np.float32), 1)
    ):
        return "causal", mval, m
    return "general", 0.0, m


def kernel(
    x, start_pos, freqs_cos, freqs_sin, mask,
    wq_w, wq_b, wk_w, wk_b, wv_w, wv_b, wo_w, wo_b,
):
    x = np.asarray(x, np.float32).reshape(S, D)
    freqs_cos = np.asarray(freqs_cos, np.float32)
    freqs_sin = np.asarray(freqs_sin, np.float32)
    mask_mode, mval, m2d = _detect_mask_mode(np.asarray(mask))

    # pair-split permutation within each head: [0,2,..,62, 1,3,..,63]
    perm1 = np.concatenate([np.arange(0, HD, 2), np.arange(1, HD, 2)])
    perm = np.concatenate([perm1 + h * HD for h in range(HPC)])

    xT_bf = np.ascontiguousarray(x.T).astype(ml_dtypes.bfloat16)

    cosT = np.ascontiguousarray(freqs_cos.T)  # [32, S]
    sinT = np.ascontiguousarray(freqs_sin.T)
    trigc = np.tile(cosT, (4, 1)).astype(np.float32)
    trigs = np.tile(sinT, (4, 1)).astype(np.float32)

    woT_full = np.ascontiguousarray(np.asarray(wo_w, np.float32).T)

    tri_np = None
    if mask_mode == "causal":
        kk = np.arange(KCH)[:, None]
        qq = np.arange(KCH)[None, :]
        tri_np = np.where(kk > qq, 0.0, 1.0).astype(ml_dtypes.bfloat16)
    maskT_np = None
    if mask_mode == "general":
        maskT_np = np.ascontiguousarray((8.0 * m2d).T.astype(np.float32))

    in_maps = []
    for c in range(NC):
        rows = slice(c * CPC, (c + 1) * CPC)
        wq_s = np.asarray(wq_w, np.float32)[rows, :][perm, :]
        wk_s = np.asarray(wk_w, np.float32)[rows, :][perm, :]
        wv_s = np.asarray(wv_w, np.float32)[rows, :]
        wqkvT = np.concatenate([wq_s.T, wk_s.T, wv_s.T], axis=1).astype(
            ml_dtypes.bfloat16
        )
        qb = np.asarray(wq_b, np.float32)[rows][perm]
        kb = np.asarray(wk_b, np.float32)[rows][perm]
        im = {
            "xT": xT_bf,
            "wqkvT": np.ascontiguousarray(wqkvT),
            "woT": np.ascontiguousarray(woT_full[rows, :]).astype(ml_dtypes.bfloat16),
            "trigc": trigc,
            "trigs": trigs,
            "qkb": np.stack([qb, kb], axis=1).astype(np.float32),
        }
        if mask_mode == "causal":
            im["tri"] = tri_np
        elif mask_mode == "general":
            im["maskT"] = maskT_np
        in_maps.append(im)

    nc = _build(mask_mode)
    res = run_bass_kernel_spmd(nc, in_maps, list(range(NC)))
    y = np.zeros((S, D), np.float64)
    for c in range(NC):
        y += np.asarray(res.results[c]["y"]).astype(np.float64)
    y += np.asarray(wo_b, np.float64)
    # v-bias folded out of the device kernel: softmax rows sum to 1, so the
    # bias contributes exactly wv_b @ wo_w.T to every output row
    y += np.asarray(wv_b, np.float64) @ np.asarray(wo_w, np.float64).T
    return y.reshape(B, S, D).astype(np.float32)


# revision 11
# speedup vs baseline: 1.3405x; 1.2885x over previous
"""Multi-head attention with RoPE (LLaMA-style) on 8 Trainium2 NeuronCores.

Head-parallel tensor parallelism: each core computes 2 of 16 heads
(projections + flash-style attention) and a partial output projection over
the full sequence; the host sums the 8 per-core partials (the all-reduce)
and adds wo_b plus the folded v-bias term.

Layout: q/k rows are pair-split per head ([evens(32)|odds(32)] per head) so
each head's QK matmul is one 64-row PE group; the two heads' matmuls run
concurrently on disjoint row groups into one [128,2,512] scores tile, and
one exp instruction covers both heads.  q/k/v projections are interleaved
with attention (PSUM: 2 proj banks + 4 scores + 2 ppv = 8); the output
projection runs as a tail with bf16 partials.

Softmax denominators come from an appended ones-column in v; reciprocal is
exp(-ln d) on the scalar engine (same activation table set as the softmax
exp), broadcast across partitions by a selection-matrix matmul.

Self-contained: hardcodes B=1, S=4096, D=1024, H=16, HD=64, 8 cores.
"""

import sys
import types

import ml_dtypes
import numpy as np

B, S, D, H, HD = 1, 4096, 1024, 16, 64
HALF = HD // 2
NC = 8                    # cores
HPC = H // NC             # heads per core (2)
CPC = HPC * HD            # qkv dims per core (128)
QCH = 512                 # query chunk (free dim of scores matmuls)
KCH = 128                 # key chunk (partition dim of scores matmuls)
NQC = S // QCH            # 8 query chunks
NKC = S // KCH            # 32 key chunks
P = 128
KC = D // P               # 8 contraction chunks for projections
VW = 2 * (HD + 1) + 2     # v_sb row width (hd|1 per head + pad, 4B aligned)


def _install_ntff_shim():
    """antenv.axon_hooks isn't injected in this image; recreate it so
    run_bass_kernel_spmd(trace=True) can capture NTFF profiles."""
    if "antenv.axon_hooks" in sys.modules:
        return
    try:
        from trn_agent_boot.trn_boot import _ntff_profile_via_ctypes

        hook = _ntff_profile_via_ctypes("/opt/axon/libaxon_pjrt.so")
    except Exception:
        hook = None
    mod = types.ModuleType("antenv.axon_hooks")
    mod.get_axon_ntff_profile_hook = lambda: hook
    sys.modules["antenv.axon_hooks"] = mod


_install_ntff_shim()

import concourse.bacc as bacc  # noqa: E402
import concourse.mybir as mybir  # noqa: E402
import concourse.tile as tile  # noqa: E402
from concourse.bass_utils import run_bass_kernel_spmd  # noqa: E402

F32 = mybir.dt.float32
BF16 = mybir.dt.bfloat16
AX = mybir.AluOpType
EXP = mybir.ActivationFunctionType.Exp
LOG = mybir.ActivationFunctionType.Ln

_BUILD_CACHE: dict = {}


def _build(mask_mode: str):
    """Build the per-core Bass program.  mask_mode: causal | none | general."""
    if mask_mode in _BUILD_CACHE:
        return _BUILD_CACHE[mask_mode]

    nc = bacc.Bacc("TRN2", target_bir_lowering=False, debug=False, num_devices=NC)

    xT = nc.dram_tensor("xT", [D, S], BF16, kind="ExternalInput")
    wqkvT = nc.dram_tensor("wqkvT", [D, 3 * CPC], BF16, kind="ExternalInput")
    woT = nc.dram_tensor("woT", [CPC, D], BF16, kind="ExternalInput")
    # trigc rows: cosT tiled 4x vertically; trigs: sinT tiled 4x
    trigc = nc.dram_tensor("trigc", [P, S], F32, kind="ExternalInput")
    trigs = nc.dram_tensor("trigs", [P, S], F32, kind="ExternalInput")
    qkb = nc.dram_tensor("qkb", [P, 2], F32, kind="ExternalInput")
    tri = None
    maskT = None
    if mask_mode == "causal":
        tri = nc.dram_tensor("tri", [KCH, KCH], BF16, kind="ExternalInput")
    elif mask_mode == "general":
        maskT = nc.dram_tensor("maskT", [S, S], F32, kind="ExternalInput")
    y_out = nc.dram_tensor("y", [S, D], BF16, kind="ExternalOutput")

    causal = mask_mode == "causal"

    def nj_of(qc):
        return 4 * (qc + 1) if causal else NKC

    with tile.TileContext(nc) as tc:
        with tc.tile_pool(name="consts", bufs=1) as cpool:
            qT = cpool.tile([P, S], BF16)   # [2 heads x (ev32|od32), s]
            kT = cpool.tile([P, S], BF16)
            v_sb = cpool.tile([P, NKC, VW], BF16)  # [s%128, s//128, hd|1 x2]
            attnT = cpool.tile([P, S], BF16)
            w_sb = cpool.tile([P, KC, 3 * CPC], BF16)
            nc.sync.dma_start(
                out=w_sb[:], in_=wqkvT.ap().rearrange("(a p) c -> p a c", p=P)
            )
            qkb_sb = cpool.tile([P, 2], F32)
            nc.sync.dma_start(out=qkb_sb[:], in_=qkb.ap())
            sel_sb = cpool.tile([33, CPC], F32)
            nc.vector.memset(sel_sb[:], 0.0)
            nc.vector.memset(sel_sb[0:1, 0:HD], 1.0)
            nc.vector.memset(sel_sb[32:33, HD:CPC], 1.0)
            nc.vector.memset(v_sb[:, :, HD : HD + 1], 1.0)
            c1 = HD + 1 + HD  # second ones column
            nc.vector.memset(v_sb[:, :, c1 : c1 + 1], 1.0)
            if causal:
                tri_sb = cpool.tile([KCH, KCH], BF16)
                nc.sync.dma_start(out=tri_sb[:], in_=tri.ap())
            woT_sb = cpool.tile([CPC, D], BF16)
            trigc_sb = cpool.tile([P, S], F32)
            trigs_sb = cpool.tile([P, S], F32)
            xT_sb = cpool.tile([P, KC, S], BF16)
            nc.sync.dma_start(
                out=xT_sb[:, :, 0:QCH],
                in_=xT.ap()[:, 0:QCH].rearrange("(a p) s -> p a s", p=P),
            )
            nc.sync.dma_start(out=trigc_sb[:], in_=trigc.ap())
            nc.sync.dma_start(out=trigs_sb[:], in_=trigs.ap())
            for sc in range(1, NQC):
                ssl = slice(sc * QCH, (sc + 1) * QCH)
                nc.sync.dma_start(
                    out=xT_sb[:, :, ssl],
                    in_=xT.ap()[:, ssl].rearrange("(a p) s -> p a s", p=P),
                )
            nc.sync.dma_start(out=woT_sb[:], in_=woT.ap())

            # ---- q/k/v projections + RoPE interleaved with attention ----
            with (
                tc.tile_pool(name="sc", bufs=2, space="PSUM") as sc_pool,
                tc.tile_pool(name="ppv", bufs=2, space="PSUM") as ppv_pool,
                tc.tile_pool(name="pt", bufs=4) as pt_pool,
                tc.tile_pool(name="tp", bufs=2) as t_pool,
                tc.tile_pool(name="dn", bufs=4) as dn_pool,
                tc.tile_pool(name="pvs", bufs=2) as pv_sb_pool,
                tc.tile_pool(name="mload", bufs=4) as mload_pool,
                tc.tile_pool(name="ysb", bufs=4) as y_pool,
            ):
                pending = []  # deferred bc+normalize closures

                def proj_units(sc):
                    """q/k/v projection + RoPE for chunk sc as small closures
                    that interleave into the attention j-loop (so the PE FIFO
                    never holds a long proj block ahead of the QK->exp chain).
                    """
                    ssl = slice(sc * QCH, (sc + 1) * QCH)
                    units = []
                    box = {}

                    def qk_mms(bcol, klo, khi, first, last):
                        def emit():
                            if first:
                                box[bcol] = pqk_pool.tile(
                                    [P, QCH], F32, tag="pqk", name="ps"
                                )
                            ps = box[bcol]
                            for kc in range(klo, khi):
                                nc.tensor.matmul(
                                    ps[:],
                                    lhsT=w_sb[
                                        :, kc, bcol * CPC : (bcol + 1) * CPC
                                    ],
                                    rhs=xT_sb[:, kc, ssl],
                                    start=(kc == 0),
                                    stop=(kc == KC - 1),
                                    skip_group_check=True,
                                )
                        return emit

                    def rope(dst, bcol):
                        def emit():
                            ps = box[bcol]
                            t = t_pool.tile([P, QCH], F32)
                            nc.vector.scalar_tensor_tensor(
                                t[:], ps[:], qkb_sb[:, bcol : bcol + 1],
                                trigc_sb[:, ssl], op0=AX.add, op1=AX.mult,
                            )
                            u = sc_pool.tile([P, QCH], F32, tag="st", name="u")
                            nc.vector.scalar_tensor_tensor(
                                u[:], ps[:], qkb_sb[:, bcol : bcol + 1],
                                trigs_sb[:, ssl], op0=AX.add, op1=AX.mult,
                            )
                            for h in range(HPC):
                                ev = slice(64 * h, 64 * h + 32)
                                od = slice(64 * h + 32, 64 * h + 64)
                                nc.vector.tensor_sub(
                                    dst[ev, ssl], t[ev, :], u[od, :]
                                )
                                nc.vector.tensor_add(
                                    dst[od, ssl], t[od, :], u[ev, :]
                                )
                        return emit

                    def v_mms(j4):
                        def emit():
                            if j4 == 0:
                                box["v"] = pqk_pool.tile(
                                    [P, 4, CPC], F32, tag="pqk", name="psv"
                                )
                            psv = box["v"]
                            sb = sc * 4 + j4
                            for kc in range(KC):
                                nc.tensor.matmul(
                                    psv[:, j4, :],
                                    lhsT=xT_sb[:, kc, sb * P : (sb + 1) * P],
                                    rhs=w_sb[:, kc, 2 * CPC : 3 * CPC],
                                    start=(j4 == 0 and kc == 0),
                                    stop=(j4 == 3 and kc == KC - 1),
                                    skip_group_check=True,
                                )
                        return emit

                    def v_copy():
                        psv = box["v"]
                        csl = slice(sc * 4, sc * 4 + 4)
                        nc.vector.tensor_copy(
                            v_sb[:, csl, 0:HD], psv[:, :, 0:HD]
                        )
                        nc.vector.tensor_copy(
                            v_sb[:, csl, HD + 1 : HD + 1 + HD], psv[:, :, HD:CPC]
                        )

                    units.append(qk_mms(0, 0, 4, True, False))
                    units.append(qk_mms(0, 4, KC, False, True))
                    units.append(qk_mms(1, 0, 4, True, False))
                    units.append(qk_mms(1, 4, KC, False, True))
                    units.append(rope(qT, 0))
                    units.append(rope(kT, 1))
                    for j4 in range(4):
                        units.append(v_mms(j4))
                    units.append(v_copy)
                    return units

                def emit_proj(sc):
                    for u in proj_units(sc):
                        u()

                def make_bc_norm(qc, pv, rec):
                    def emit():
                        qsl = slice(qc * QCH, (qc + 1) * QCH)
                        bc_ps = sc_pool.tile([P, QCH], F32, tag="st", name="bc")
                        nc.tensor.matmul(
                            bc_ps[:], lhsT=sel_sb[:], rhs=rec[:],
                            start=True, stop=True, skip_group_check=True,
                        )
                        for h in range(HPC):
                            hr = slice(64 * h, 64 * h + 64)
                            nc.vector.tensor_mul(
                                attnT[hr, qsl], pv[hr, :], bc_ps[hr, :]
                            )

                    return emit

                def emit_attn(qc, units=()):
                    units = list(units)
                    nj = nj_of(qc)
                    q0 = qc * QCH
                    ppv0 = ppv_pool.tile([HD + 1, QCH], F32, tag="ppv", name="ppv0")
                    ppv1 = ppv_pool.tile([HD + 1, QCH], F32, tag="ppv", name="ppv1")
                    prevs = []

                    def emit_pv(ent):
                        j, pt, lo = ent
                        for h, ppv in ((0, ppv0), (1, ppv1)):
                            nc.tensor.matmul(
                                ppv[:, lo:QCH],
                                lhsT=v_sb[
                                    :, j, h * (HD + 1) : (h + 1) * (HD + 1)
                                ],
                                rhs=pt[:, h, lo:QCH],
                                start=(j == 0),
                                stop=(j == nj - 1),
                                skip_group_check=True,
                            )

                    for j in range(nj):
                        lo = max(0, KCH * j - q0) if causal else 0
                        diag = causal and KCH * j >= q0
                        st = sc_pool.tile([P, 2, QCH], F32, tag="st", name="st")
                        for h in range(HPC):
                            hr = slice(64 * h, 64 * h + 64)
                            nc.tensor.matmul(
                                st[:, h, lo:QCH],
                                lhsT=kT[hr, j * KCH : (j + 1) * KCH],
                                rhs=qT[hr, q0 + lo : q0 + QCH],
                                start=True,
                                stop=True,
                                skip_group_check=True,
                            )
                        if units and j >= 1:
                            npop = 3 if nj < 8 else 2
                            for _ in range(npop):
                                if units:
                                    units.pop(0)()
                        if j == 3 and pending:
                            pending.pop(0)()
                        if len(prevs) == 3:
                            emit_pv(prevs.pop(0))
                        if mask_mode == "general":
                            mt = mload_pool.tile([KCH, QCH], F32)
                            nc.sync.dma_start(
                                out=mt[:],
                                in_=maskT.ap()[
                                    j * KCH : (j + 1) * KCH, q0 : q0 + QCH
                                ],
                            )
                            for h in range(HPC):
                                nc.vector.tensor_add(
                                    st[:, h, :], st[:, h, :], mt[:]
                                )
                        pt = pt_pool.tile([P, 2, QCH], BF16)
                        nc.scalar.activation(
                            pt[:, :, lo:QCH], st[:, :, lo:QCH], EXP, scale=0.125
                        )
                        if diag:
                            # multiplicative 0/1 mask after the exp: cheap
                            # (bf16 2x) and two j-periods off the PV chain
                            for h in range(HPC):
                                nc.gpsimd.tensor_mul(
                                    pt[:, h, lo : lo + KCH],
                                    pt[:, h, lo : lo + KCH],
                                    tri_sb[:],
                                )
                        prevs.append((j, pt, lo))
                    for u in units:
                        u()
                    for ent in prevs:
                        emit_pv(ent)

                    # normalization prep; reciprocal as exp(-ln d) on the
                    # scalar engine (same table set as the softmax exp)
                    dn = dn_pool.tile([33, QCH], F32, tag="dn", name="dn")
                    nc.gpsimd.memset(dn[:], 1.0)
                    nc.vector.tensor_copy(dn[0:1, :], ppv0[HD : HD + 1, :])
                    nc.vector.tensor_copy(dn[32:33, :], ppv1[HD : HD + 1, :])
                    pv = pv_sb_pool.tile([P, QCH], F32)
                    nc.vector.tensor_copy(pv[0:HD, :], ppv0[0:HD, :])
                    nc.vector.tensor_copy(pv[HD:P, :], ppv1[0:HD, :])
                    rec = dn_pool.tile([33, QCH], F32, tag="rec", name="rec")
                    nc.vector.reciprocal(rec[:], dn[:])
                    pending.append(make_bc_norm(qc, pv, rec))

                def make_wo(py_pool, sb):
                    def emit():
                        ysb = y_pool.tile([P, 2, QCH], BF16, tag="y", name="ysb")
                        for dh in range(2):
                            psy = py_pool.tile(
                                [P, QCH], F32, tag="py", name="psy"
                            )
                            nc.tensor.matmul(
                                psy[:],
                                lhsT=attnT[:, sb * P : (sb + 1) * P],
                                rhs=woT_sb[:, dh * QCH : (dh + 1) * QCH],
                                start=True,
                                stop=True,
                                skip_group_check=True,
                            )
                            nc.vector.tensor_copy(ysb[:, dh, :], psy[:])
                        nc.sync.dma_start(
                            out=y_out.ap()[sb * P : (sb + 1) * P, :],
                            in_=ysb[:],
                        )

                    return emit

                if causal:
                    with tc.tile_pool(
                        name="pqk", bufs=2, space="PSUM"
                    ) as pqk_pool:
                        emit_proj(0)
                        for qc in range(NQC - 1):
                            emit_attn(qc, proj_units(qc + 1))
                    # proj psum banks are free now: overlap the first 28
                    # output-projection row-blocks with the last (largest)
                    # attention step, then finish the rest as a short tail
                    with tc.tile_pool(
                        name="py", bufs=2, space="PSUM"
                    ) as py_pool:
                        wo_units = [make_wo(py_pool, sb) for sb in range(28)]
                        emit_attn(NQC - 1, wo_units)
                        for fn in pending:
                            fn()
                        pending.clear()
                        for sb in range(28, 32):
                            make_wo(py_pool, sb)()
                else:
                    with tc.tile_pool(
                        name="pqk", bufs=2, space="PSUM"
                    ) as pqk_pool:
                        for sc in range(NQC):
                            emit_proj(sc)
                        for qc in range(NQC):
                            emit_attn(qc)
                    with tc.tile_pool(
                        name="py", bufs=2, space="PSUM"
                    ) as py_pool:
                        for fn in pending:
                            fn()
                        pending.clear()
                        for sb in range(32):
                            make_wo(py_pool, sb)()


    nc.compile()
    _BUILD_CACHE[mask_mode] = nc
    return nc


def _detect_mask_mode(mask: np.ndarray):
    m = np.asarray(mask, np.float32).reshape(S, S)
    if not m.any():
        return "none", 0.0, m
    mval = float(m[0, 1])
    if mval < -1e8 and np.array_equal(
        m, np.triu(np.full((S, S), mval, np.float32), 1)
    ):
        return "causal", mval, m
    return "general", 0.0, m


def kernel(
    x, start_pos, freqs_cos, freqs_sin, mask,
    wq_w, wq_b, wk_w, wk_b, wv_w, wv_b, wo_w, wo_b,
):
    x = np.asarray(x, np.float32).reshape(S, D)
    freqs_cos = np.asarray(freqs_cos, np.float32)
    freqs_sin = np.asarray(freqs_sin, np.float32)
    mask_mode, mval, m2d = _detect_mask_mode(np.asarray(mask))

    # pair-split permutation within each head: [0,2,..,62, 1,3,..,63]
    perm1 = np.concatenate([np.arange(0, HD, 2), np.arange(1, HD, 2)])
    perm = np.concatenate([perm1 + h * HD for h in range(HPC)])

    xT_bf = np.ascontiguousarray(x.T).astype(ml_dtypes.bfloat16)

    cosT = np.ascontiguousarray(freqs_cos.T)  # [32, S]
    sinT = np.ascontiguousarray(freqs_sin.T)
    trigc = np.tile(cosT, (4, 1)).astype(np.float32)
    trigs = np.tile(sinT, (4, 1)).astype(np.float32)

    woT_full = np.ascontiguousarray(np.asarray(wo_w, np.float32).T)

    tri_np = None
    if mask_mode == "causal":
        kk = np.arange(KCH)[:, None]
        qq = np.arange(KCH)[None, :]
        tri_np = np.where(kk > qq, 0.0, 1.0).astype(ml_dtypes.bfloat16)
    maskT_np = None
    if mask_mode == "general":
        maskT_np = np.ascontiguousarray((8.0 * m2d).T.astype(np.float32))

    in_maps = []
    for c in range(NC):
        rows = slice(c * CPC, (c + 1) * CPC)
        wq_s = np.asarray(wq_w, np.float32)[rows, :][perm, :]
        wk_s = np.asarray(wk_w, np.float32)[rows, :][perm, :]
        wv_s = np.asarray(wv_w, np.float32)[rows, :]
        wqkvT = np.concatenate([wq_s.T, wk_s.T, wv_s.T], axis=1).astype(
            ml_dtypes.bfloat16
        )
        qb = np.asarray(wq_b, np.float32)[rows][perm]
        kb = np.asarray(wk_b, np.float32)[rows][perm]
        im = {
            "xT": xT_bf,
            "wqkvT": np.ascontiguousarray(wqkvT),
            "woT": np.ascontiguousarray(woT_full[rows, :]).astype(ml_dtypes.bfloat16),
            "trigc": trigc,
            "trigs": trigs,
            "qkb": np.stack([qb, kb], axis=1).astype(np.float32),
        }
        if mask_mode == "causal":
            im["tri"] = tri_np
        elif mask_mode == "general":
            im["maskT"] = maskT_np
        in_maps.append(im)

    nc = _build(mask_mode)
    res = run_bass_kernel_spmd(nc, in_maps, list(range(NC)))
    y = np.zeros((S, D), np.float64)
    for c in range(NC):
        y += np.asarray(res.results[c]["y"]).astype(np.float64)
    y += np.asarray(wo_b, np.float64)
    # v-bias folded out of the device kernel: softmax rows sum to 1, so the
    # bias contributes exactly wv_b @ wo_w.T to every output row
    y += np.asarray(wv_b, np.float64) @ np.asarray(wo_w, np.float64).T
    return y.reshape(B, S, D).astype(np.float32)


# revision 12
# speedup vs baseline: 1.3672x; 1.0199x over previous
"""Multi-head attention with RoPE (LLaMA-style) on 8 Trainium2 NeuronCores.

Head-parallel tensor parallelism: each core computes 2 of 16 heads
(projections + flash-style attention) and a partial output projection over
the full sequence; the host sums the 8 per-core partials (the all-reduce)
and adds wo_b plus the folded v-bias term.

Layout: q/k rows are pair-split per head ([evens(32)|odds(32)] per head) so
each head's QK matmul is one 64-row PE group; the two heads' matmuls run
concurrently on disjoint row groups into one [128,2,512] scores tile, and
one exp instruction covers both heads.  q/k/v projections are interleaved
with attention (PSUM: 2 proj banks + 4 scores + 2 ppv = 8); the output
projection runs as a tail with bf16 partials.

Softmax denominators come from an appended ones-column in v; reciprocal is
exp(-ln d) on the scalar engine (same activation table set as the softmax
exp), broadcast across partitions by a selection-matrix matmul.

Self-contained: hardcodes B=1, S=4096, D=1024, H=16, HD=64, 8 cores.
"""

import sys
import types

import ml_dtypes
import numpy as np

B, S, D, H, HD = 1, 4096, 1024, 16, 64
HALF = HD // 2
NC = 8                    # cores
HPC = H // NC             # heads per core (2)
CPC = HPC * HD            # qkv dims per core (128)
QCH = 512                 # query chunk (free dim of scores matmuls)
KCH = 128                 # key chunk (partition dim of scores matmuls)
NQC = S // QCH            # 8 query chunks
NKC = S // KCH            # 32 key chunks
P = 128
KC = D // P               # 8 contraction chunks for projections
VW = 2 * (HD + 1) + 2     # v_sb row width (hd|1 per head + pad, 4B aligned)


def _install_ntff_shim():
    """antenv.axon_hooks isn't injected in this image; recreate it so
    run_bass_kernel_spmd(trace=True) can capture NTFF profiles."""
    if "antenv.axon_hooks" in sys.modules:
        return
    try:
        from trn_agent_boot.trn_boot import _ntff_profile_via_ctypes

        hook = _ntff_profile_via_ctypes("/opt/axon/libaxon_pjrt.so")
    except Exception:
        hook = None
    mod = types.ModuleType("antenv.axon_hooks")
    mod.get_axon_ntff_profile_hook = lambda: hook
    sys.modules["antenv.axon_hooks"] = mod


_install_ntff_shim()

import concourse.bacc as bacc  # noqa: E402
import concourse.mybir as mybir  # noqa: E402
import concourse.tile as tile  # noqa: E402
from concourse.bass_utils import run_bass_kernel_spmd  # noqa: E402

F32 = mybir.dt.float32
BF16 = mybir.dt.bfloat16
AX = mybir.AluOpType
EXP = mybir.ActivationFunctionType.Exp
LOG = mybir.ActivationFunctionType.Ln

_BUILD_CACHE: dict = {}


def _build(mask_mode: str):
    """Build the per-core Bass program.  mask_mode: causal | none | general."""
    if mask_mode in _BUILD_CACHE:
        return _BUILD_CACHE[mask_mode]

    nc = bacc.Bacc("TRN2", target_bir_lowering=False, debug=False, num_devices=NC)

    xT = nc.dram_tensor("xT", [D, S], BF16, kind="ExternalInput")
    wqkvT = nc.dram_tensor("wqkvT", [D, 3 * CPC], BF16, kind="ExternalInput")
    woT = nc.dram_tensor("woT", [CPC, D], BF16, kind="ExternalInput")
    # trigc rows: cosT tiled 4x vertically; trigs: sinT tiled 4x
    trigc = nc.dram_tensor("trigc", [P, S], F32, kind="ExternalInput")
    trigs = nc.dram_tensor("trigs", [P, S], F32, kind="ExternalInput")
    qkb = nc.dram_tensor("qkb", [P, 2], F32, kind="ExternalInput")
    tri = None
    maskT = None
    if mask_mode == "causal":
        tri = nc.dram_tensor("tri", [KCH, KCH], BF16, kind="ExternalInput")
    elif mask_mode == "general":
        maskT = nc.dram_tensor("maskT", [S, S], F32, kind="ExternalInput")
    y_out = nc.dram_tensor("y", [S, D], BF16, kind="ExternalOutput")

    causal = mask_mode == "causal"

    def nj_of(qc):
        return 4 * (qc + 1) if causal else NKC

    with tile.TileContext(nc) as tc:
        with tc.tile_pool(name="consts", bufs=1) as cpool:
            qT = cpool.tile([P, S], BF16)   # [2 heads x (ev32|od32), s]
            kT = cpool.tile([P, S], BF16)
            v_sb = cpool.tile([P, NKC, VW], BF16)  # [s%128, s//128, hd|1 x2]
            attnT = cpool.tile([P, S], BF16)
            w_sb = cpool.tile([P, KC, 3 * CPC], BF16)
            nc.sync.dma_start(
                out=w_sb[:], in_=wqkvT.ap().rearrange("(a p) c -> p a c", p=P)
            )
            qkb_sb = cpool.tile([P, 2], F32)
            nc.sync.dma_start(out=qkb_sb[:], in_=qkb.ap())
            sel_sb = cpool.tile([33, CPC], F32)
            nc.vector.memset(sel_sb[:], 0.0)
            nc.vector.memset(sel_sb[0:1, 0:HD], 1.0)
            nc.vector.memset(sel_sb[32:33, HD:CPC], 1.0)
            nc.vector.memset(v_sb[:, :, HD : HD + 1], 1.0)
            c1 = HD + 1 + HD  # second ones column
            nc.vector.memset(v_sb[:, :, c1 : c1 + 1], 1.0)
            if causal:
                tri_sb = cpool.tile([KCH, KCH], BF16)
                nc.sync.dma_start(out=tri_sb[:], in_=tri.ap())
            woT_sb = cpool.tile([CPC, D], BF16)
            trigc_sb = cpool.tile([P, S], F32)
            trigs_sb = cpool.tile([P, S], F32)
            xT_sb = cpool.tile([P, KC, S], BF16)
            nc.sync.dma_start(
                out=xT_sb[:, :, 0:QCH],
                in_=xT.ap()[:, 0:QCH].rearrange("(a p) s -> p a s", p=P),
            )
            nc.sync.dma_start(out=trigc_sb[:], in_=trigc.ap())
            nc.sync.dma_start(out=trigs_sb[:], in_=trigs.ap())
            for sc in range(1, NQC):
                ssl = slice(sc * QCH, (sc + 1) * QCH)
                nc.sync.dma_start(
                    out=xT_sb[:, :, ssl],
                    in_=xT.ap()[:, ssl].rearrange("(a p) s -> p a s", p=P),
                )
            nc.sync.dma_start(out=woT_sb[:], in_=woT.ap())

            # ---- q/k/v projections + RoPE interleaved with attention ----
            with (
                tc.tile_pool(name="sc", bufs=2, space="PSUM") as sc_pool,
                tc.tile_pool(name="ppv", bufs=2, space="PSUM") as ppv_pool,
                tc.tile_pool(name="pt", bufs=4) as pt_pool,
                tc.tile_pool(name="tp", bufs=2) as t_pool,
                tc.tile_pool(name="dn", bufs=4) as dn_pool,
                tc.tile_pool(name="pvs", bufs=2) as pv_sb_pool,
                tc.tile_pool(name="mload", bufs=4) as mload_pool,
                tc.tile_pool(name="ysb", bufs=4) as y_pool,
            ):
                pending = []  # deferred bc+normalize closures

                def proj_units(sc):
                    """q/k/v projection + RoPE for chunk sc as small closures
                    that interleave into the attention j-loop (so the PE FIFO
                    never holds a long proj block ahead of the QK->exp chain).
                    """
                    ssl = slice(sc * QCH, (sc + 1) * QCH)
                    units = []
                    box = {}

                    def qk_mms(bcol, klo, khi, first, last):
                        def emit():
                            if first:
                                box[bcol] = pqk_pool.tile(
                                    [P, QCH], F32, tag="pqk", name="ps"
                                )
                            ps = box[bcol]
                            for kc in range(klo, khi):
                                nc.tensor.matmul(
                                    ps[:],
                                    lhsT=w_sb[
                                        :, kc, bcol * CPC : (bcol + 1) * CPC
                                    ],
                                    rhs=xT_sb[:, kc, ssl],
                                    start=(kc == 0),
                                    stop=(kc == KC - 1),
                                    skip_group_check=True,
                                )
                        return emit

                    def rope(dst, bcol):
                        def emit():
                            ps = box[bcol]
                            t = t_pool.tile([P, QCH], F32)
                            nc.vector.scalar_tensor_tensor(
                                t[:], ps[:], qkb_sb[:, bcol : bcol + 1],
                                trigc_sb[:, ssl], op0=AX.add, op1=AX.mult,
                            )
                            u = sc_pool.tile([P, QCH], F32, tag="st", name="u")
                            nc.vector.scalar_tensor_tensor(
                                u[:], ps[:], qkb_sb[:, bcol : bcol + 1],
                                trigs_sb[:, ssl], op0=AX.add, op1=AX.mult,
                            )
                            for h in range(HPC):
                                ev = slice(64 * h, 64 * h + 32)
                                od = slice(64 * h + 32, 64 * h + 64)
                                nc.vector.tensor_sub(
                                    dst[ev, ssl], t[ev, :], u[od, :]
                                )
                                nc.vector.tensor_add(
                                    dst[od, ssl], t[od, :], u[ev, :]
                                )
                        return emit

                    def v_mms(j4):
                        def emit():
                            if j4 == 0:
                                box["v"] = pqk_pool.tile(
                                    [P, 4, CPC], F32, tag="pqk", name="psv"
                                )
                            psv = box["v"]
                            sb = sc * 4 + j4
                            for kc in range(KC):
                                nc.tensor.matmul(
                                    psv[:, j4, :],
                                    lhsT=xT_sb[:, kc, sb * P : (sb + 1) * P],
                                    rhs=w_sb[:, kc, 2 * CPC : 3 * CPC],
                                    start=(j4 == 0 and kc == 0),
                                    stop=(j4 == 3 and kc == KC - 1),
                                    skip_group_check=True,
                                )
                        return emit

                    def v_copy():
                        psv = box["v"]
                        csl = slice(sc * 4, sc * 4 + 4)
                        nc.vector.tensor_copy(
                            v_sb[:, csl, 0:HD], psv[:, :, 0:HD]
                        )
                        nc.vector.tensor_copy(
                            v_sb[:, csl, HD + 1 : HD + 1 + HD], psv[:, :, HD:CPC]
                        )

                    units.append(qk_mms(0, 0, 4, True, False))
                    units.append(qk_mms(0, 4, KC, False, True))
                    units.append(rope(qT, 0))
                    units.append(qk_mms(1, 0, 4, True, False))
                    units.append(qk_mms(1, 4, KC, False, True))
                    units.append(rope(kT, 1))
                    for j4 in range(4):
                        units.append(v_mms(j4))
                    units.append(v_copy)
                    return [(sc, u) for u in units]

                def emit_proj(sc):
                    for _, u in proj_units(sc):
                        u()

                def make_bc_norm(qc, pv, rec):
                    def emit():
                        qsl = slice(qc * QCH, (qc + 1) * QCH)
                        bc_ps = sc_pool.tile([P, QCH], F32, tag="st", name="bc")
                        nc.tensor.matmul(
                            bc_ps[:], lhsT=sel_sb[:], rhs=rec[:],
                            start=True, stop=True, skip_group_check=True,
                        )
                        for h in range(HPC):
                            hr = slice(64 * h, 64 * h + 64)
                            nc.vector.tensor_mul(
                                attnT[hr, qsl], pv[hr, :], bc_ps[hr, :]
                            )

                    return emit

                work = []  # global (sc, closure) queue, 2-step lookahead

                def emit_attn(qc, units=()):
                    units = work
                    nj = nj_of(qc)
                    q0 = qc * QCH
                    ppv0 = ppv_pool.tile([HD + 1, QCH], F32, tag="ppv", name="ppv0")
                    ppv1 = ppv_pool.tile([HD + 1, QCH], F32, tag="ppv", name="ppv1")
                    prevs = []

                    def emit_pv(ent):
                        j, pt, lo = ent
                        for h, ppv in ((0, ppv0), (1, ppv1)):
                            nc.tensor.matmul(
                                ppv[:, lo:QCH],
                                lhsT=v_sb[
                                    :, j, h * (HD + 1) : (h + 1) * (HD + 1)
                                ],
                                rhs=pt[:, h, lo:QCH],
                                start=(j == 0),
                                stop=(j == nj - 1),
                                skip_group_check=True,
                            )

                    for j in range(nj):
                        lo = max(0, KCH * j - q0) if causal else 0
                        diag = causal and KCH * j >= q0
                        st = sc_pool.tile([P, 2, QCH], F32, tag="st", name="st")
                        for h in range(HPC):
                            hr = slice(64 * h, 64 * h + 64)
                            nc.tensor.matmul(
                                st[:, h, lo:QCH],
                                lhsT=kT[hr, j * KCH : (j + 1) * KCH],
                                rhs=qT[hr, q0 + lo : q0 + QCH],
                                start=True,
                                stop=True,
                                skip_group_check=True,
                            )
                        if units:
                            npop = 3 if nj < 8 else 2
                            for _ in range(npop):
                                if units:
                                    units.pop(0)[1]()
                        if j == 3 and pending:
                            pending.pop(0)()
                        if len(prevs) == 3:
                            emit_pv(prevs.pop(0))
                        if mask_mode == "general":
                            mt = mload_pool.tile([KCH, QCH], F32)
                            nc.sync.dma_start(
                                out=mt[:],
                                in_=maskT.ap()[
                                    j * KCH : (j + 1) * KCH, q0 : q0 + QCH
                                ],
                            )
                            for h in range(HPC):
                                nc.vector.tensor_add(
                                    st[:, h, :], st[:, h, :], mt[:]
                                )
                        pt = pt_pool.tile([P, 2, QCH], BF16)
                        nc.scalar.activation(
                            pt[:, :, lo:QCH], st[:, :, lo:QCH], EXP, scale=0.125
                        )
                        if diag:
                            # multiplicative 0/1 mask after the exp: cheap
                            # (bf16 2x) and two j-periods off the PV chain
                            for h in range(HPC):
                                nc.gpsimd.tensor_mul(
                                    pt[:, h, lo : lo + KCH],
                                    pt[:, h, lo : lo + KCH],
                                    tri_sb[:],
                                )
                        prevs.append((j, pt, lo))
                    while units and units[0][0] <= qc + 1:
                        units.pop(0)[1]()
                    for ent in prevs:
                        emit_pv(ent)

                    # normalization prep; reciprocal as exp(-ln d) on the
                    # scalar engine (same table set as the softmax exp)
                    dn = dn_pool.tile([33, QCH], F32, tag="dn", name="dn")
                    nc.gpsimd.memset(dn[:], 1.0)
                    nc.vector.tensor_copy(dn[0:1, :], ppv0[HD : HD + 1, :])
                    nc.vector.tensor_copy(dn[32:33, :], ppv1[HD : HD + 1, :])
                    pv = pv_sb_pool.tile([P, QCH], F32)
                    nc.vector.tensor_copy(pv[0:HD, :], ppv0[0:HD, :])
                    nc.vector.tensor_copy(pv[HD:P, :], ppv1[0:HD, :])
                    rec = dn_pool.tile([33, QCH], F32, tag="rec", name="rec")
                    nc.vector.reciprocal(rec[:], dn[:])
                    pending.append(make_bc_norm(qc, pv, rec))

                def make_wo(py_pool, sb):
                    def emit():
                        ysb = y_pool.tile([P, 2, QCH], BF16, tag="y", name="ysb")
                        for dh in range(2):
                            psy = py_pool.tile(
                                [P, QCH], F32, tag="py", name="psy"
                            )
                            nc.tensor.matmul(
                                psy[:],
                                lhsT=attnT[:, sb * P : (sb + 1) * P],
                                rhs=woT_sb[:, dh * QCH : (dh + 1) * QCH],
                                start=True,
                                stop=True,
                                skip_group_check=True,
                            )
                            nc.vector.tensor_copy(ysb[:, dh, :], psy[:])
                        nc.sync.dma_start(
                            out=y_out.ap()[sb * P : (sb + 1) * P, :],
                            in_=ysb[:],
                        )

                    return emit

                if causal:
                    with tc.tile_pool(
                        name="pqk", bufs=2, space="PSUM"
                    ) as pqk_pool:
                        emit_proj(0)
                        work.extend(proj_units(1))
                        for qc in range(NQC - 1):
                            if qc + 2 < NQC:
                                work.extend(proj_units(qc + 2))
                            emit_attn(qc)
                    # proj psum banks are free now: overlap the first 28
                    # output-projection row-blocks with the last (largest)
                    # attention step, then finish the rest as a short tail
                    with tc.tile_pool(
                        name="py", bufs=2, space="PSUM"
                    ) as py_pool:
                        work.extend(
                            (NQC, make_wo(py_pool, sb)) for sb in range(28)
                        )
                        emit_attn(NQC - 1)
                        while work:
                            work.pop(0)[1]()
                        for fn in pending:
                            fn()
                        pending.clear()
                        for sb in range(28, 32):
                            make_wo(py_pool, sb)()
                else:
                    with tc.tile_pool(
                        name="pqk", bufs=2, space="PSUM"
                    ) as pqk_pool:
                        for sc in range(NQC):
                            emit_proj(sc)
                        for qc in range(NQC):
                            emit_attn(qc)
                    with tc.tile_pool(
                        name="py", bufs=2, space="PSUM"
                    ) as py_pool:
                        for fn in pending:
                            fn()
                        pending.clear()
                        for sb in range(32):
                            make_wo(py_pool, sb)()


    nc.compile()
    _BUILD_CACHE[mask_mode] = nc
    return nc


def _detect_mask_mode(mask: np.ndarray):
    m = np.asarray(mask, np.float32).reshape(S, S)
    if not m.any():
        return "none", 0.0, m
    mval = float(m[0, 1])
    if mval < -1e8 and np.array_equal(
        m, np.triu(np.full((S, S), mval, np.float32), 1)
    ):
        return "causal", mval, m
    return "general", 0.0, m


def kernel(
    x, start_pos, freqs_cos, freqs_sin, mask,
    wq_w, wq_b, wk_w, wk_b, wv_w, wv_b, wo_w, wo_b,
):
    x = np.asarray(x, np.float32).reshape(S, D)
    freqs_cos = np.asarray(freqs_cos, np.float32)
    freqs_sin = np.asarray(freqs_sin, np.float32)
    mask_mode, mval, m2d = _detect_mask_mode(np.asarray(mask))

    # pair-split permutation within each head: [0,2,..,62, 1,3,..,63]
    perm1 = np.concatenate([np.arange(0, HD, 2), np.arange(1, HD, 2)])
    perm = np.concatenate([perm1 + h * HD for h in range(HPC)])

    xT_bf = np.ascontiguousarray(x.T).astype(ml_dtypes.bfloat16)

    cosT = np.ascontiguousarray(freqs_cos.T)  # [32, S]
    sinT = np.ascontiguousarray(freqs_sin.T)
    trigc = np.tile(cosT, (4, 1)).astype(np.float32)
    trigs = np.tile(sinT, (4, 1)).astype(np.float32)

    woT_full = np.ascontiguousarray(np.asarray(wo_w, np.float32).T)

    tri_np = None
    if mask_mode == "causal":
        kk = np.arange(KCH)[:, None]
        qq = np.arange(KCH)[None, :]
        tri_np = np.where(kk > qq, 0.0, 1.0).astype(ml_dtypes.bfloat16)
    maskT_np = None
    if mask_mode == "general":
        maskT_np = np.ascontiguousarray((8.0 * m2d).T.astype(np.float32))

    in_maps = []
    for c in range(NC):
        rows = slice(c * CPC, (c + 1) * CPC)
        wq_s = np.asarray(wq_w, np.float32)[rows, :][perm, :]
        wk_s = np.asarray(wk_w, np.float32)[rows, :][perm, :]
        wv_s = np.asarray(wv_w, np.float32)[rows, :]
        wqkvT = np.concatenate([wq_s.T, wk_s.T, wv_s.T], axis=1).astype(
            ml_dtypes.bfloat16
        )
        qb = np.asarray(wq_b, np.float32)[rows][perm]
        kb = np.asarray(wk_b, np.float32)[rows][perm]
        im = {
            "xT": xT_bf,
            "wqkvT": np.ascontiguousarray(wqkvT),
            "woT": np.ascontiguousarray(woT_full[rows, :]).astype(ml_dtypes.bfloat16),
            "trigc": trigc,
            "trigs": trigs,
            "qkb": np.stack([qb, kb], axis=1).astype(np.float32),
        }
        if mask_mode == "causal":
            im["tri"] = tri_np
        elif mask_mode == "general":
            im["maskT"] = maskT_np
        in_maps.append(im)

    nc = _build(mask_mode)
    res = run_bass_kernel_spmd(nc, in_maps, list(range(NC)))
    y = np.zeros((S, D), np.float64)
    for c in range(NC):
        y += np.asarray(res.results[c]["y"]).astype(np.float64)
    y += np.asarray(wo_b, np.float64)
    # v-bias folded out of the device kernel: softmax rows sum to 1, so the
    # bias contributes exactly wv_b @ wo_w.T to every output row
    y += np.asarray(wv_b, np.float64) @ np.asarray(wo_w, np.float64).T
    return y.reshape(B, S, D).astype(np.float32)
